# revision 21
# baseline (speedup 1.0000x reference)
"""Distributed blocked-cumprod kernel for Trainium2 (8 NeuronCores).

Problem: alpha_bars = cumprod(1 - betas) over T = 2**25 f32 elements.

Strategy (parallel-scan decomposition, pipelined over global chunks):
  - T is split into NCHUNKS global chunks; each chunk is sharded
    contiguously across the 8 cores.  Core k's kernel input is the
    concatenation of its NCHUNKS pieces.
  - Per chunk, per core: view the piece as [128 x cols] row-major.
    ScalarE computes alpha = 1 - beta in place, VectorE runs a chained
    tensor_tensor_scan (op0=mult) -> per-partition local cumprods.
    VectorE runs ONLY these scans (plus a tiny clamp and a share of the
    last chunk's scaling) so nothing queues behind them.
  - All prefix plumbing runs in LOG space on ScalarE + PE so the
    post-collective path never touches VectorE:
      row totals -> clamp -> Ln -> [strict-lower-tri matmul] row
      prefix logs; core-total log -> tiny AllGather of 8 logs ->
      per-core mask matmul -> exclusive core prefix log + chunk total
      log; running cross-chunk carry log added on ScalarE; broadcast
      add via K=1 matmul into the same PSUM; Exp -> per-partition
      scale factor.
  - ScalarE (VectorE assists on the last chunk) multiplies each tile
    by its prefix in place; DMA out.  Chunk c's exchange + scale +
    store overlap chunk c+1's load + scan.
"""

import sys

sys.path.insert(0, "/opt/trn_rl_repo")

import numpy as np

from concourse import bacc, mybir, tile
from concourse.bass_utils import run_bass_kernel_spmd

NCORES = 8
P = 128
T_FULL = 33554432
NCHUNKS = 3

_F32 = mybir.dt.float32

# aux constant layout: [128, 259]
#   [:, 0:128]   strict lower-tri (as lhsT: aux[k, m] = 1 iff k < m)
#   [:, 128:256] all-ones block (ones column / ones row views)
#   [0:8, 256]   per-core exclusive mask (1 for ranks < mine)
#   [0:8, 257]   all-ones mask
#   [0:8, 258]   per-core onehot (1 for my rank)
_AUX_COLS = 259


def _tile_widths(cols: int, first_chunk: bool, last_chunk: bool, max_w: int = 2048):
    """Column widths of the scan tiles for one chunk."""
    widths = []
    tail = []
    if first_chunk and cols >= 2 * max_w:
        # small leading tiles so the first scan starts early
        widths = [max_w // 8, max_w // 8, max_w // 4, max_w // 2]
        cols -= sum(widths)
    if last_chunk and cols >= 2 * max_w:
        # small trailing tiles so the final scale+store tail is short
        tail = [max_w // 2, max_w // 4, max_w // 8, max_w // 8]
        cols -= sum(tail)
    while cols > 0:
        w = min(max_w, cols)
        widths.append(w)
        cols -= w
    return widths + tail


def _cols_split(total_cols: int, nchunks: int, max_w: int):
    q = max((total_cols // nchunks) // max_w * max_w, 1)
    rem = total_cols - q * (nchunks - 1)
    return [rem] + [q] * (nchunks - 1)


def build_nc(shard_len: int, nchunks: int, max_w: int = 2048):
    total_cols = shard_len // P
    assert shard_len == P * total_cols
    cols_split = _cols_split(total_cols, nchunks, max_w)
    chunk_offs = [P * sum(cols_split[:i]) for i in range(nchunks + 1)]

    mult = mybir.AluOpType.mult
    bypass = mybir.AluOpType.bypass
    identity = mybir.ActivationFunctionType.Identity
    copyf = mybir.ActivationFunctionType.Copy
    lnf = mybir.ActivationFunctionType.Ln
    expf = mybir.ActivationFunctionType.Exp

    nc = bacc.Bacc(num_devices=NCORES)
    betas = nc.declare_dram_parameter("betas", [shard_len], _F32, isOutput=False)
    aux_in = nc.declare_dram_parameter("aux", [P, _AUX_COLS], _F32, isOutput=False)
    out = nc.declare_dram_parameter("out", [shard_len], _F32, isOutput=True)

    with tile.TileContext(nc) as tc:
        with (
            tc.tile_pool(name="data", bufs=1) as data_pool,
            tc.tile_pool(name="c0data", bufs=1) as c0_pool,
            tc.tile_pool(name="small", bufs=1) as small_pool,
            tc.tile_pool(name="psum", bufs=2, space="PSUM") as psum_pool,
            tc.tile_pool(name="dram", bufs=1, space="DRAM") as dram_pool,
        ):
            aux = small_pool.tile([P, _AUX_COLS], _F32, name="aux")
            nc.sync.dma_start(aux[:], aux_in[:, :])
            lstrict = aux[:, 0:P]
            ones_col = aux[:, P : P + 1]
            ones_row = aux[0:1, P : 2 * P]
            maskcols = aux[0:NCORES, 2 * P : 2 * P + 3]

            carrylog = [
                small_pool.tile([1, 1], _F32, name=f"carrylog{c}")
                for c in range(nchunks + 1)
            ]
            nc.scalar.memzero(carrylog[0][:])

            # Warm-up barrier: absorbs cross-core launch skew and the
            # first-collective setup cost off the critical path.
            wu_sb = small_pool.tile([1, 1], _F32, name="wu_sb")
            nc.scalar.memzero(wu_sb[:])
            wu_in = dram_pool.tile([1, 1], _F32, name="wu_in")
            wu_out = dram_pool.tile([NCORES, 1], _F32, name="wu_out", addr_space="Shared")
            nc.sync.dma_start(wu_in[:], wu_sb[:])
            nc.gpsimd.collective_compute(
                "AllGather",
                mybir.AluOpType.bypass,
                replica_groups=[list(range(NCORES))],
                ins=[wu_in.opt()],
                outs=[wu_out.opt()],
            )

            for c in range(nchunks):
                cols = cols_split[c]
                bview = betas.ap()[chunk_offs[c] : chunk_offs[c + 1]].rearrange(
                    "(p c) -> p c", p=P
                )
                oview = out.ap()[chunk_offs[c] : chunk_offs[c + 1]].rearrange(
                    "(p c) -> p c", p=P
                )
                widths = _tile_widths(cols, first_chunk=(c == 0), last_chunk=(c == nchunks - 1), max_w=max_w)
                offs = [sum(widths[:j]) for j in range(len(widths))]
                nt = len(widths)

                if c == 0:
                    A = [
                        c0_pool.tile([P, widths[j]], _F32, name=f"c0t{j}")
                        for j in range(nt)
                    ]
                else:
                    A = [
                        data_pool.tile([P, widths[j]], _F32, name=f"a{c}_{j}")
                        for j in range(nt)
                    ]

                # Phase 1: load, alpha = 1 - beta, chained local scan.
                for j in range(nt):
                    nc.sync.dma_start(A[j][:], bview[:, offs[j] : offs[j] + widths[j]])
                for j in range(nt):
                    nc.scalar.activation(A[j][:], A[j][:], identity, bias=1.0, scale=-1.0)
                for j in range(nt):
                    init = 1.0 if j == 0 else A[j - 1][:, widths[j - 1] - 1 :]
                    nc.vector.tensor_tensor_scan(A[j][:], A[j][:], A[j][:], init, mult, bypass)

                # Hint ladder (order-only, per chunk):
                #   h_post(c)   post-collective smalls (sc_log/fprefix/gath)
                #   h_pre(c+1)  next chunk's trigger path (Ln/ct/lnct)
                #   h_scale(c)  the long scale+store group
                # This keeps every in-order engine queue from blocking a
                # trigger path or the 1-x/scan stream behind an op that
                # waits on a (skew-delayed) collective.
                h_post = 0.070 + 0.012 * c
                h_pre = h_post + 0.001
                h_scale = h_post + 0.002

                # Trigger path: clamp (V) -> Ln (S) -> core-total matmul (PE)
                # -> copy (S) -> 4-byte bounce DMA (gpsimd) -> doorbell.
                pre_ms = None if c == 0 else (0.070 + 0.012 * (c - 1) + 0.001)
                with tc.tile_wait_until(pre_ms if pre_ms is not None else 0, enable=pre_ms is not None):
                    rowtotc = small_pool.tile([P, 1], _F32, name=f"rowtotc{c}")
                    nc.vector.tensor_scalar_max(
                        rowtotc[:], A[nt - 1][:, widths[nt - 1] - 1 :], 1e-38
                    )
                    lnrt = small_pool.tile([P, 1], _F32, name=f"lnrt{c}")
                    nc.scalar.activation(lnrt[:], rowtotc[:], lnf)
                    ct_ps = psum_pool.tile([1, 1], _F32, name="ct_ps")
                    nc.tensor.matmul(ct_ps[:], lnrt[:], ones_col, start=True, stop=True)
                    lnct = small_pool.tile([1, 1], _F32, name=f"lnct{c}")
                    nc.scalar.copy(lnct[:], ct_ps[:])

                cc_in = dram_pool.tile([1, 1], _F32, name=f"cc_in{c}")
                cc_out = dram_pool.tile(
                    [NCORES, 1], _F32, name=f"cc_out{c}", addr_space="Shared"
                )
                nc.gpsimd.dma_start(cc_in[:], lnct[:])
                nc.gpsimd.collective_compute(
                    "AllGather",
                    bypass,
                    replica_groups=[list(range(NCORES))],
                    ins=[cc_in.opt()],
                    outs=[cc_out.opt()],
                )

                with tc.tile_wait_until(h_post):
                    gathp = small_pool.tile([NCORES, 1], _F32, name=f"gathp{c}")
                    nc.sync.dma_start(gathp[:], cc_out[:, :])
                    # [cpref_log, chunktot_log] = gathered^T @ [excl, ones]
                    pc_ps = psum_pool.tile([1, 2], _F32, name="pc_ps")
                    nc.tensor.matmul(
                        pc_ps[:], gathp[:], maskcols[:, 0:2], start=True, stop=True
                    )
                    sc_log = small_pool.tile([1, 1], _F32, name=f"sc_log{c}")
                    nc.scalar.activation(
                        sc_log[:], pc_ps[0:1, 0:1], identity, bias=carrylog[c][:]
                    )
                    if c + 1 < nchunks:
                        nc.scalar.activation(
                            carrylog[c + 1][:], pc_ps[0:1, 1:2], identity, bias=carrylog[c][:]
                        )
                    # fprefix = exp(strict-lower-tri @ lnrt + sc_log)
                    fp_ps = psum_pool.tile([P, 1], _F32, name="fp_ps")
                    nc.tensor.matmul(fp_ps[:], lstrict, lnrt[:], start=True, stop=False)
                    nc.tensor.matmul(fp_ps[:], ones_row, sc_log[:], start=False, stop=True)
                    fprefix = small_pool.tile([P, 1], _F32, name=f"fprefix{c}")
                    nc.scalar.activation(fprefix[:], fp_ps[:], expf)

                with tc.tile_wait_until(h_scale):
                    # Phase 3: scale in place (split ScalarE/VectorE), store.
                    for j in range(nt):
                        if j % 2 == 1:
                            nc.vector.tensor_scalar_mul(A[j][:], A[j][:], fprefix[:])
                        else:
                            nc.scalar.activation(A[j][:], A[j][:], copyf, scale=fprefix[:])
                        nc.sync.dma_start(oview[:, offs[j] : offs[j] + widths[j]], A[j][:])

    nc.compile()
    return nc


def _make_aux(core: int) -> np.ndarray:
    aux = np.zeros((P, _AUX_COLS), dtype=np.float32)
    aux[:, 0:P] = np.triu(np.ones((P, P), np.float32), k=1)  # aux[k,m]=1 iff k<m
    aux[:, P : 2 * P] = 1.0
    aux[0:core, 2 * P] = 1.0
    aux[0:NCORES, 2 * P + 1] = 1.0
    aux[core, 2 * P + 2] = 1.0
    return aux


def _shard_slices(total: int, max_w: int = 2048):
    """Per-core (global range, local range) pairs, one per chunk."""
    total_cols = total // (NCORES * P)
    cols_split = _cols_split(total_cols, NCHUNKS, max_w)
    out = []
    for k in range(NCORES):
        pairs = []
        goff = 0
        loff = 0
        for c in range(NCHUNKS):
            chunk = cols_split[c] * P * NCORES
            piece = cols_split[c] * P
            pairs.append(((goff + k * piece, goff + (k + 1) * piece), (loff, loff + piece)))
            goff += chunk
            loff += piece
        out.append(pairs)
    return out


def make_in_maps(betas: np.ndarray, max_w: int = 2048):
    slices = _shard_slices(betas.size, max_w)
    in_maps = []
    for k in range(NCORES):
        shard = np.concatenate([betas[a:b] for (a, b), _ in slices[k]])
        in_maps.append({"betas": shard, "aux": _make_aux(k)})
    return in_maps


def assemble(results, total: int, max_w: int = 2048) -> np.ndarray:
    out = np.empty(total, dtype=np.float32)
    slices = _shard_slices(total, max_w)
    for k in range(NCORES):
        shard = results[k]["out"]
        for (a, b), (la, lb) in slices[k]:
            out[a:b] = shard[la:lb]
    return out


def kernel(betas: np.ndarray) -> np.ndarray:
    betas = np.asarray(betas, dtype=np.float32).reshape(-1)
    assert betas.size == T_FULL, betas.size
    nc = build_nc(T_FULL // NCORES, NCHUNKS)
    in_maps = make_in_maps(betas)
    res = run_bass_kernel_spmd(nc, in_maps, core_ids=list(range(NCORES)))
    return assemble(res.results, T_FULL)


# revision 22
# speedup vs baseline: 1.1148x; 1.1148x over previous
"""Distributed blocked-cumprod kernel for Trainium2 (8 NeuronCores).

Problem: alpha_bars = cumprod(1 - betas) over T = 2**25 f32 elements.

Strategy (parallel-scan decomposition, pipelined over global chunks):
  - T is split into NCHUNKS global chunks; each chunk is sharded
    contiguously across the 8 cores.  Core k's kernel input is the
    concatenation of its NCHUNKS pieces.
  - Per chunk, per core: view the piece as [128 x cols] row-major.
    ScalarE computes alpha = 1 - beta in place, VectorE runs a chained
    tensor_tensor_scan (op0=mult) -> per-partition local cumprods.
    VectorE runs ONLY these scans (plus a tiny clamp and a share of the
    last chunk's scaling) so nothing queues behind them.
  - All prefix plumbing runs in LOG space on ScalarE + PE so the
    post-collective path never touches VectorE:
      row totals -> clamp -> Ln -> [strict-lower-tri matmul] row
      prefix logs; core-total log -> tiny AllGather of 8 logs ->
      per-core mask matmul -> exclusive core prefix log + chunk total
      log; running cross-chunk carry log added on ScalarE; broadcast
      add via K=1 matmul into the same PSUM; Exp -> per-partition
      scale factor.
  - ScalarE (VectorE assists on the last chunk) multiplies each tile
    by its prefix in place; DMA out.  Chunk c's exchange + scale +
    store overlap chunk c+1's load + scan.
"""

import sys

sys.path.insert(0, "/opt/trn_rl_repo")

import numpy as np

from concourse import bacc, mybir, tile
from concourse.bass_utils import run_bass_kernel_spmd

NCORES = 8
P = 128
T_FULL = 33554432
NCHUNKS = 2

_F32 = mybir.dt.float32

# aux constant layout: [128, 259]
#   [:, 0:128]   strict lower-tri (as lhsT: aux[k, m] = 1 iff k < m)
#   [:, 128:256] all-ones block (ones column / ones row views)
#   [0:8, 256]   per-core exclusive mask (1 for ranks < mine)
#   [0:8, 257]   all-ones mask
#   [0:8, 258]   per-core onehot (1 for my rank)
_AUX_COLS = 259


def _tile_widths(cols: int, first_chunk: bool, last_chunk: bool, max_w: int = 2048):
    """Column widths of the scan tiles for one chunk."""
    widths = []
    tail = []
    if first_chunk and cols >= 2 * max_w:
        # small leading tiles so the first scan starts early
        widths = [max_w // 8, max_w // 8, max_w // 4, max_w // 2]
        cols -= sum(widths)
    if last_chunk and cols >= 2 * max_w:
        # small trailing tiles so the final scale+store tail is short
        tail = [max_w // 2, max_w // 4, max_w // 8, max_w // 8]
        cols -= sum(tail)
    while cols > 0:
        w = min(max_w, cols)
        widths.append(w)
        cols -= w
    return widths + tail


def _cols_split(total_cols: int, nchunks: int, max_w: int):
    q = max((total_cols // nchunks) // max_w * max_w, 1)
    rem = total_cols - q * (nchunks - 1)
    return [rem] + [q] * (nchunks - 1)


def build_nc(shard_len: int, nchunks: int, max_w: int = 2048):
    total_cols = shard_len // P
    assert shard_len == P * total_cols
    cols_split = _cols_split(total_cols, nchunks, max_w)
    chunk_offs = [P * sum(cols_split[:i]) for i in range(nchunks + 1)]

    mult = mybir.AluOpType.mult
    bypass = mybir.AluOpType.bypass
    identity = mybir.ActivationFunctionType.Identity
    copyf = mybir.ActivationFunctionType.Copy
    lnf = mybir.ActivationFunctionType.Ln
    expf = mybir.ActivationFunctionType.Exp

    nc = bacc.Bacc(num_devices=NCORES)
    betas = nc.declare_dram_parameter("betas", [shard_len], _F32, isOutput=False)
    aux_in = nc.declare_dram_parameter("aux", [P, _AUX_COLS], _F32, isOutput=False)
    out = nc.declare_dram_parameter("out", [shard_len], _F32, isOutput=True)

    with tile.TileContext(nc) as tc:
        with (
            tc.tile_pool(name="data", bufs=1) as data_pool,
            tc.tile_pool(name="c0data", bufs=1) as c0_pool,
            tc.tile_pool(name="small", bufs=1) as small_pool,
            tc.tile_pool(name="psum", bufs=2, space="PSUM") as psum_pool,
            tc.tile_pool(name="dram", bufs=1, space="DRAM") as dram_pool,
        ):
            aux = small_pool.tile([P, _AUX_COLS], _F32, name="aux")
            nc.sync.dma_start(aux[:], aux_in[:, :])
            lstrict = aux[:, 0:P]
            ones_col = aux[:, P : P + 1]
            ones_row = aux[0:1, P : 2 * P]
            maskcols = aux[0:NCORES, 2 * P : 2 * P + 3]

            carrylog = [
                small_pool.tile([1, 1], _F32, name=f"carrylog{c}")
                for c in range(nchunks + 1)
            ]
            nc.scalar.memzero(carrylog[0][:])

            # Warm-up barrier: absorbs cross-core launch skew and the
            # first-collective setup cost off the critical path.
            wu_sb = small_pool.tile([1, 1], _F32, name="wu_sb")
            nc.scalar.memzero(wu_sb[:])
            wu_in = dram_pool.tile([1, 1], _F32, name="wu_in")
            wu_out = dram_pool.tile([NCORES, 1], _F32, name="wu_out", addr_space="Shared")
            nc.sync.dma_start(wu_in[:], wu_sb[:])
            nc.gpsimd.collective_compute(
                "AllGather",
                mybir.AluOpType.bypass,
                replica_groups=[list(range(NCORES))],
                ins=[wu_in.opt()],
                outs=[wu_out.opt()],
            )

            for c in range(nchunks):
                cols = cols_split[c]
                bview = betas.ap()[chunk_offs[c] : chunk_offs[c + 1]].rearrange(
                    "(p c) -> p c", p=P
                )
                oview = out.ap()[chunk_offs[c] : chunk_offs[c + 1]].rearrange(
                    "(p c) -> p c", p=P
                )
                widths = _tile_widths(cols, first_chunk=(c == 0), last_chunk=(c == nchunks - 1), max_w=max_w)
                offs = [sum(widths[:j]) for j in range(len(widths))]
                nt = len(widths)

                if c == 0:
                    A = [
                        c0_pool.tile([P, widths[j]], _F32, name=f"c0t{j}")
                        for j in range(nt)
                    ]
                else:
                    A = [
                        data_pool.tile([P, widths[j]], _F32, name=f"a{c}_{j}")
                        for j in range(nt)
                    ]

                # Phase 1: load, alpha = 1 - beta, chained local scan.
                for j in range(nt):
                    nc.sync.dma_start(A[j][:], bview[:, offs[j] : offs[j] + widths[j]])
                for j in range(nt):
                    nc.scalar.activation(A[j][:], A[j][:], identity, bias=1.0, scale=-1.0)
                for j in range(nt):
                    init = 1.0 if j == 0 else A[j - 1][:, widths[j - 1] - 1 :]
                    nc.vector.tensor_tensor_scan(A[j][:], A[j][:], A[j][:], init, mult, bypass)

                # Hint ladder (order-only, per chunk):
                #   h_post(c)   post-collective smalls (sc_log/fprefix/gath)
                #   h_pre(c+1)  next chunk's trigger path (Ln/ct/lnct)
                #   h_scale(c)  the long scale+store group
                # This keeps every in-order engine queue from blocking a
                # trigger path or the 1-x/scan stream behind an op that
                # waits on a (skew-delayed) collective.
                h_post = 0.070 + 0.012 * c
                h_pre = h_post + 0.001
                h_scale = h_post + 0.002

                # Trigger path: clamp (V) -> Ln (S) -> core-total matmul (PE)
                # -> copy (S) -> 4-byte bounce DMA (gpsimd) -> doorbell.
                pre_ms = None if c == 0 else (0.070 + 0.012 * (c - 1) + 0.001)
                with tc.tile_wait_until(pre_ms if pre_ms is not None else 0, enable=pre_ms is not None):
                    rowtotc = small_pool.tile([P, 1], _F32, name=f"rowtotc{c}")
                    nc.vector.tensor_scalar_max(
                        rowtotc[:], A[nt - 1][:, widths[nt - 1] - 1 :], 1e-38
                    )
                    lnrt = small_pool.tile([P, 1], _F32, name=f"lnrt{c}")
                    nc.scalar.activation(lnrt[:], rowtotc[:], lnf)
                    ct_ps = psum_pool.tile([1, 1], _F32, name="ct_ps")
                    nc.tensor.matmul(ct_ps[:], lnrt[:], ones_col, start=True, stop=True)
                    lnct = small_pool.tile([1, 1], _F32, name=f"lnct{c}")
                    nc.scalar.copy(lnct[:], ct_ps[:])

                cc_in = dram_pool.tile([1, 1], _F32, name=f"cc_in{c}")
                cc_out = dram_pool.tile(
                    [NCORES, 1], _F32, name=f"cc_out{c}", addr_space="Shared"
                )
                nc.gpsimd.dma_start(cc_in[:], lnct[:])
                nc.gpsimd.collective_compute(
                    "AllGather",
                    bypass,
                    replica_groups=[list(range(NCORES))],
                    ins=[cc_in.opt()],
                    outs=[cc_out.opt()],
                )

                with tc.tile_wait_until(h_post):
                    gathp = small_pool.tile([NCORES, 1], _F32, name=f"gathp{c}")
                    nc.sync.dma_start(gathp[:], cc_out[:, :])
                    # [cpref_log, chunktot_log] = gathered^T @ [excl, ones]
                    pc_ps = psum_pool.tile([1, 2], _F32, name="pc_ps")
                    nc.tensor.matmul(
                        pc_ps[:], gathp[:], maskcols[:, 0:2], start=True, stop=True
                    )
                    sc_log = small_pool.tile([1, 1], _F32, name=f"sc_log{c}")
                    nc.scalar.activation(
                        sc_log[:], pc_ps[0:1, 0:1], identity, bias=carrylog[c][:]
                    )
                    if c + 1 < nchunks:
                        nc.scalar.activation(
                            carrylog[c + 1][:], pc_ps[0:1, 1:2], identity, bias=carrylog[c][:]
                        )
                    # fprefix = exp(strict-lower-tri @ lnrt + sc_log)
                    fp_ps = psum_pool.tile([P, 1], _F32, name="fp_ps")
                    nc.tensor.matmul(fp_ps[:], lstrict, lnrt[:], start=True, stop=False)
                    nc.tensor.matmul(fp_ps[:], ones_row, sc_log[:], start=False, stop=True)
                    fprefix = small_pool.tile([P, 1], _F32, name=f"fprefix{c}")
                    nc.scalar.activation(fprefix[:], fp_ps[:], expf)

                with tc.tile_wait_until(h_scale):
                    # Phase 3: scale in place (split ScalarE/VectorE), store.
                    for j in range(nt):
                        if j % 2 == 1:
                            nc.vector.tensor_scalar_mul(A[j][:], A[j][:], fprefix[:])
                        else:
                            nc.scalar.activation(A[j][:], A[j][:], copyf, scale=fprefix[:])
                        nc.sync.dma_start(oview[:, offs[j] : offs[j] + widths[j]], A[j][:])

    nc.compile()
    return nc


def _make_aux(core: int) -> np.ndarray:
    aux = np.zeros((P, _AUX_COLS), dtype=np.float32)
    aux[:, 0:P] = np.triu(np.ones((P, P), np.float32), k=1)  # aux[k,m]=1 iff k<m
    aux[:, P : 2 * P] = 1.0
    aux[0:core, 2 * P] = 1.0
    aux[0:NCORES, 2 * P + 1] = 1.0
    aux[core, 2 * P + 2] = 1.0
    return aux


def _shard_slices(total: int, max_w: int = 2048):
    """Per-core (global range, local range) pairs, one per chunk."""
    total_cols = total // (NCORES * P)
    cols_split = _cols_split(total_cols, NCHUNKS, max_w)
    out = []
    for k in range(NCORES):
        pairs = []
        goff = 0
        loff = 0
        for c in range(NCHUNKS):
            chunk = cols_split[c] * P * NCORES
            piece = cols_split[c] * P
            pairs.append(((goff + k * piece, goff + (k + 1) * piece), (loff, loff + piece)))
            goff += chunk
            loff += piece
        out.append(pairs)
    return out


def make_in_maps(betas: np.ndarray, max_w: int = 2048):
    slices = _shard_slices(betas.size, max_w)
    in_maps = []
    for k in range(NCORES):
        shard = np.concatenate([betas[a:b] for (a, b), _ in slices[k]])
        in_maps.append({"betas": shard, "aux": _make_aux(k)})
    return in_maps


def assemble(results, total: int, max_w: int = 2048) -> np.ndarray:
    out = np.empty(total, dtype=np.float32)
    slices = _shard_slices(total, max_w)
    for k in range(NCORES):
        shard = results[k]["out"]
        for (a, b), (la, lb) in slices[k]:
            out[a:b] = shard[la:lb]
    return out


def kernel(betas: np.ndarray) -> np.ndarray:
    betas = np.asarray(betas, dtype=np.float32).reshape(-1)
    assert betas.size == T_FULL, betas.size
    nc = build_nc(T_FULL // NCORES, NCHUNKS)
    in_maps = make_in_maps(betas)
    res = run_bass_kernel_spmd(nc, in_maps, core_ids=list(range(NCORES)))
    return assemble(res.results, T_FULL)
